# revision 23
# baseline (speedup 1.0000x reference)
"""Multi-head self-attention TRN2 kernel.

Full inputs -> shard over 8 NeuronCores as (batch b, head-group g):
core c = 2*b + g handles batch b and heads 8g..8g+7 (tensor parallel over
heads within a batch entry). Each core computes its heads' contribution to
the output projection; the host sums the two partials per batch and adds
proj bias.

Per-core pipeline (all matmuls bf16 with fp32 PSUM accumulation):
  x^T   via transposing DMA                      [C=1024, T=2048]
  Q^T,K^T = (w.T as lhsT) @ x^T  (+bias, DVE)    [512, T] col-major
  V     = (x^T as lhsT) @ wv (+bias via K=1 mm)  [T, 512] + ones col
  per head-pair hp, t-chunk of 1024, s-tile of 128:
    S^T chunk = K^T.T @ Q^T      (K=64 contraction, 2 heads row-tiled)
    P^T = exp(S^T/8)             ScalarE from PSUM, bf16 out, N=1024
    O  += P^T.T @ V_aug          (per 128-t subtile, N=65; col 64 = rowsum)
  normalize O by 1/rowsum (per-partition scalar), DMA-transpose -> O^T
  partial = O^T.T @ pw           [T, 1024] fp32 -> DRAM
"""

import numpy as np
import ml_dtypes
from contextlib import ExitStack

import concourse.bass as bass
import concourse.bacc as bacc
import concourse.tile as tile
from concourse import mybir
from concourse.bass_utils import run_bass_kernel_spmd

BF16 = mybir.dt.bfloat16
F32 = mybir.dt.float32
bf16 = ml_dtypes.bfloat16

P = 128
C = 1024          # hidden
HG = 8            # heads per core
D = 64            # head dim
DG = HG * D       # 512, per-core qkv width
N_CORES = 8
FULL_T = 2048
SCALE = D ** -0.5


def build_kernel(T=FULL_T):
    nc = bacc.Bacc(
        "TRN2", target_bir_lowering=False, debug=False, num_devices=N_CORES
    )
    x = nc.dram_tensor("x", [T, C], BF16, kind="ExternalInput").ap()
    wq = nc.dram_tensor("wq", [P, C // P, DG], BF16, kind="ExternalInput").ap()
    wk = nc.dram_tensor("wk", [P, C // P, DG], BF16, kind="ExternalInput").ap()
    wv = nc.dram_tensor("wv", [P, C // P, DG], BF16, kind="ExternalInput").ap()
    # cols 0..3 = q bias per col-tile, 4..7 = k bias
    bqk = nc.dram_tensor("bqk", [P, 8], F32, kind="ExternalInput").ap()
    bv = nc.dram_tensor("bv", [1, DG], BF16, kind="ExternalInput").ap()
    pw = nc.dram_tensor("pw", [P, DG // P, C], BF16, kind="ExternalInput").ap()
    partial = nc.dram_tensor("partial", [T, C], BF16, kind="ExternalOutput").ap()

    CT = C // P           # 8 contraction tiles over hidden
    TT = T // P           # t/s tiles of 128
    TCH = min(1024, T)    # t chunk width for attention (exp granularity)
    NCH = T // TCH        # number of t chunks
    SW = min(512, TCH)    # matmul moving-dim width (PSUM bank limit)
    NSW = TCH // SW       # sub-chunks per chunk
    Q8 = TCH // P         # 128-t subtiles per chunk
    KT4 = DG // P         # 4 col-tiles of Q^T/K^T/O^T

    with tile.TileContext(nc) as tc, ExitStack() as ctx:
        sb = ctx.enter_context(tc.tile_pool(name="sb", bufs=1))
        pdram = ctx.enter_context(tc.tile_pool(name="pdram", bufs=4, space="DRAM"))
        pon = ctx.enter_context(tc.tile_pool(name="pon", bufs=3))
        ppb = ctx.enter_context(tc.tile_pool(name="ppb", bufs=16))
        pout = ctx.enter_context(tc.tile_pool(name="pout", bufs=2))
        pp = ctx.enter_context(tc.tile_pool(name="pp", bufs=1, space="PSUM"))

        def o_tag(hx):
            return ("oA", "oB")[hx]

        # persistent SBUF tensors
        xT = sb.tile([P, CT, T], BF16)
        wq_s = sb.tile([P, CT, DG], BF16)
        wk_s = sb.tile([P, CT, DG], BF16)
        wv_s = sb.tile([P, CT, DG], BF16)
        pw_s = sb.tile([P, KT4, C], BF16)
        bqk_s = sb.tile([P, 8], F32)
        bvb_s = sb.tile([P, DG], BF16)
        QT = sb.tile([P, KT4, T], BF16)
        KTt = sb.tile([P, KT4, T], BF16)
        V = sb.tile([P, TT, HG, 65], BF16)
        OT = sb.tile([P, KT4, T], BF16)

        # ---- loads ----
        # ALL transposing DMAs on one queue (concurrent xbar-mode DMAs on
        # the two HWDGE queues corrupt data — verified empirically), in
        # half-T chunks so the th=0 half lands early and compute can start
        # while the th=1 half streams. Weights on the other queue, K/Q
        # weights first since they gate the first attention pass.
        for q in range(T // SW):
            for ct in range(CT):
                nc.sync.dma_start(
                    out=xT[:, ct, q * SW : (q + 1) * SW],
                    in_=x[q * SW : (q + 1) * SW, ct * P : (ct + 1) * P],
                    transpose=True,
                )
        nc.scalar.dma_start(out=wk_s, in_=wk)
        nc.scalar.dma_start(out=wq_s, in_=wq)
        nc.scalar.dma_start(out=bqk_s, in_=bqk)
        nc.scalar.dma_start(out=wv_s, in_=wv)
        nc.gpsimd.dma_start(out=bvb_s, in_=bv.to_broadcast((P, DG)))
        nc.gpsimd.dma_start(out=pw_s, in_=pw)
        nc.vector.memset(V[:, :, :, 64:65], 1.0)

        # ---- helpers ----
        # Fill work (V/QK/proj tiles) lives on a dedicated 2-bank psum tag
        # so it never blocks the attention chain's banks.
        def v_tile(tt):
            ps = pp.tile([P, DG], F32, tag="fill", name="psv")
            for ct in range(CT):
                nc.tensor.matmul(
                    ps,
                    lhsT=xT[:, ct, tt * P : (tt + 1) * P],
                    rhs=wv_s[:, ct, :],
                    start=(ct == 0),
                    stop=(ct == CT - 1),
                )
            # bias via DVE broadcast-add fused into the PSUM->SBUF copy
            nc.vector.tensor_add(
                V[:, tt, :, 0:64], ps.rearrange("p (h d) -> p h d", h=HG),
                bvb_s.rearrange("p (h d) -> p h d", h=HG),
            )

        def qk_tile(w_s, QKT, boff, i, th, tag="fill"):
            ps = pp.tile([P, TCH], F32, tag=tag, name="ps")
            for ct in range(CT):
                for nh in range(NSW):
                    nc.tensor.matmul(
                        ps[:, nh * SW : (nh + 1) * SW],
                        lhsT=w_s[:, ct, i * P : (i + 1) * P],
                        rhs=xT[
                            :, ct, th * TCH + nh * SW : th * TCH + (nh + 1) * SW
                        ],
                        start=(ct == 0),
                        stop=(ct == CT - 1),
                    )
            nc.vector.tensor_scalar_add(
                QKT[:, i, th * TCH : (th + 1) * TCH],
                ps,
                bqk_s[:, boff + i : boff + i + 1],
            )

        def proj_chunk(th, tags=("fill",)):
            for mt in range(th * (TT // NCH), (th + 1) * (TT // NCH)):
                ps_p = pp.tile([P, C], F32, tag=tags[mt % len(tags)], name="ps_p")
                for kk in range(KT4):
                    for nh in range(C // 512):
                        nc.tensor.matmul(
                            ps_p[:, nh * 512 : (nh + 1) * 512],
                            lhsT=OT[:, kk, mt * P : (mt + 1) * P],
                            rhs=pw_s[:, kk, nh * 512 : (nh + 1) * 512],
                            start=(kk == 0),
                            stop=(kk == KT4 - 1),
                        )
                ot = pout.tile([P, C], BF16, tag="ot", name="ot")
                nc.vector.tensor_copy(ot, ps_p)
                eng = nc.sync if mt % 2 == 0 else nc.scalar
                eng.dma_start(out=partial[mt * P : (mt + 1) * P, :], in_=ot)

        # ---- attention: single-stream software pipeline over 16 passes ----
        # One head-pass at a time; the S tiles double-buffer across two
        # psum tags (sA/sB by global step parity) so ScalarE never waits:
        # while exp(step) runs, PE computes S(step+1). PSUM: sA, sB
        # ([128, TCH] = 2 banks each) + oA accumulator ([65, TCH] = 2) = 6
        # banks; the remaining pair ("fill", 2 banks) hosts V/QK/proj tiles
        # which the scheduler slots into PE slack without ever touching the
        # chain's banks.
        passes = [
            (hp, th, hx)
            for hp in range(KT4)
            for th in range(NCH)
            for hx in range(2)
        ]

        def s_mm(hp, th, hx, st, parity):
            s_ps = pp.tile([P, TCH], F32, tag=("sA", "sB")[parity], name="s_ps")
            pr = slice(hx * 64, (hx + 1) * 64)
            for nh in range(NSW):
                nc.tensor.matmul(
                    s_ps[:, nh * SW : (nh + 1) * SW],
                    lhsT=KTt[pr, hp, st * P : (st + 1) * P],
                    rhs=QT[
                        pr, hp, th * TCH + nh * SW : th * TCH + (nh + 1) * SW
                    ],
                    start=True,
                    stop=True,
                )
            return s_ps

        def normalize(o_t, hp, th, hx):
            # copy O_aug^T out of PSUM first so the bank frees fast
            ou = pon.tile([65, TCH], F32, tag="ou", name="ou")
            nc.vector.tensor_copy(ou, o_t)
            eng0, eng1 = nc.sync, nc.scalar
            # 1/rowsum: bounce the [1,TCH] rowsum through DRAM into a
            # [128, TCH/128] layout so DVE reciprocal uses all lanes (a
            # [1,TCH] reciprocal costs 6.5us on one lane), then bounce
            # back -> stride-0 broadcast to 64 partitions
            rd = pdram.tile([1, TCH], F32, tag="rd", name="rd")
            eng0.dma_start(out=rd, in_=ou[64:65, :])
            rs = pon.tile([P, TCH // P], F32, tag="rs", name="rs")
            eng0.dma_start(
                out=rs, in_=rd.rearrange("a (p k) -> (a p) k", p=P)
            )
            rq = pon.tile([P, TCH // P], F32, tag="rq", name="rq")
            nc.vector.reciprocal(rq, rs)
            rqd = pdram.tile([1, TCH], F32, tag="rqd", name="rqd")
            eng1.dma_start(
                out=rqd.rearrange("a (p k) -> (a p) k", p=P), in_=rq
            )
            rb = pon.tile([64, TCH], F32, tag="rb", name="rb")
            eng1.dma_start(out=rb, in_=rqd.to_broadcast((64, TCH)))
            nc.vector.tensor_mul(
                OT[hx * 64 : (hx + 1) * 64, hp, th * TCH : (th + 1) * TCH],
                ou[0:64, :],
                rb,
            )

        # ---- emission ----
        # Fill work first (program order = dependency order: the chain
        # reads V/QT/KTt), interleaved in rough consumption order. The
        # chain itself is emitted afterwards under high_priority so the
        # scheduler prefers chain instructions whenever they are ready and
        # only slots fill work into PE idle moments.
        with tc.high_priority(offset=2 * 10**7):
            qk_tile(wk_s, KTt, 4, 0, 0)          # th0-half data only
            qk_tile(wq_s, QT, 0, 0, 0, tag="sB")  # parallel with KT0th0
        qk_tile(wk_s, KTt, 4, 0, 1)      # pass 0 needs s-tiles 8..15
        v_tile(0)
        v_tile(1)
        v_tile(2)
        v_tile(3)
        v_tile(4)
        v_tile(5)
        v_tile(6)
        v_tile(7)
        v_tile(8)
        v_tile(9)
        v_tile(10)
        v_tile(11)
        v_tile(12)
        v_tile(13)
        v_tile(14)
        v_tile(15)
        qk_tile(wq_s, QT, 0, 0, 1)       # passes 2-3 (hp=0, th=1) step 32
        qk_tile(wk_s, KTt, 4, 1, 0)      # passes 4-7 (hp=1) at step 64
        qk_tile(wk_s, KTt, 4, 1, 1)
        qk_tile(wq_s, QT, 0, 1, 0)
        qk_tile(wq_s, QT, 0, 1, 1)
        qk_tile(wk_s, KTt, 4, 2, 0)      # passes 8-11 (hp=2) at step 128
        qk_tile(wk_s, KTt, 4, 2, 1)
        qk_tile(wq_s, QT, 0, 2, 0)
        qk_tile(wq_s, QT, 0, 2, 1)
        qk_tile(wk_s, KTt, 4, 3, 0)      # passes 12-15 (hp=3) at step 192
        qk_tile(wk_s, KTt, 4, 3, 1)
        qk_tile(wq_s, QT, 0, 3, 0)
        qk_tile(wq_s, QT, 0, 3, 1)

        with tc.high_priority(offset=10**7):
            step = 0
            s_cur = s_mm(*passes[0], 0, 0)
            for pi, (hp, th, hx) in enumerate(passes):
                h = 2 * hp + hx
                o_t = pp.tile([65, TCH], F32, tag="oA", name="o_t")
                for st in range(TT):
                    pb = ppb.tile([P, TCH], BF16, tag="p", name="pb")
                    nc.scalar.activation(
                        out=pb,
                        in_=s_cur,
                        func=mybir.ActivationFunctionType.Exp,
                        scale=float(SCALE),
                    )
                    # O_aug^T += V_aug.T @ P^T (V stationary, P streams)
                    for nh in range(NSW):
                        nc.tensor.matmul(
                            o_t[:, nh * SW : (nh + 1) * SW],
                            lhsT=V[:, st, h, :],
                            rhs=pb[:, nh * SW : (nh + 1) * SW],
                            start=(st == 0),
                            stop=(st == TT - 1),
                        )
                    # prefetch next step's S (next st, or next pass's st=0)
                    step += 1
                    if st + 1 < TT:
                        s_cur = s_mm(hp, th, hx, st + 1, step % 2)
                    elif pi + 1 < len(passes):
                        s_cur = s_mm(*passes[pi + 1], 0, step % 2)
                normalize(o_t, hp, th, hx)

        proj_chunk(0, tags=("fill", "sA", "sB", "oA"))
        proj_chunk(1, tags=("fill", "sA", "sB", "oA"))

    nc.compile()
    return nc


def shard_inputs(x, qkv_w, qkv_b, proj_w, proj_b, T=FULL_T):
    """Build the 8 per-core input maps (host-side layout prep)."""
    x = np.asarray(x, dtype=np.float32)
    qkv_w = np.asarray(qkv_w, dtype=np.float32)
    qkv_b = np.asarray(qkv_b, dtype=np.float32)
    proj_w = np.asarray(proj_w, dtype=np.float32)
    in_maps = []
    for c in range(N_CORES):
        b, g = divmod(c, 2)
        sl = slice(g * DG, (g + 1) * DG)
        wqg = qkv_w[:, 0 * C + g * DG : 0 * C + (g + 1) * DG]
        wkg = qkv_w[:, 1 * C + g * DG : 1 * C + (g + 1) * DG]
        wvg = qkv_w[:, 2 * C + g * DG : 2 * C + (g + 1) * DG]
        bqg = qkv_b[0 * C + g * DG : 0 * C + (g + 1) * DG]
        bkg = qkv_b[1 * C + g * DG : 1 * C + (g + 1) * DG]
        bvg = qkv_b[2 * C + g * DG : 2 * C + (g + 1) * DG]
        pwg = proj_w[sl, :]

        def arr_w(w):  # [C, DG] -> [128, C//128, DG]
            return np.ascontiguousarray(
                w.reshape(C // P, P, DG).transpose(1, 0, 2)
            ).astype(bf16)

        bqk = np.ascontiguousarray(
            np.concatenate(
                [bqg.reshape(DG // P, P).T, bkg.reshape(DG // P, P).T], axis=1
            )
        ).astype(np.float32)
        in_maps.append(
            {
                "x": np.ascontiguousarray(x[b, :T]).astype(bf16),
                "wq": arr_w(wqg),
                "wk": arr_w(wkg),
                "wv": arr_w(wvg),
                "bqk": bqk,
                "bv": np.ascontiguousarray(bvg[None, :]).astype(bf16),
                "pw": np.ascontiguousarray(
                    pwg.reshape(DG // P, P, C).transpose(1, 0, 2)
                ).astype(bf16),
            }
        )
    return in_maps


def combine_outputs(results, proj_b, T=FULL_T):
    proj_b = np.asarray(proj_b, dtype=np.float32)
    out = np.empty((N_CORES // 2, T, C), np.float32)
    for b in range(N_CORES // 2):
        out[b] = (
            results[2 * b]["partial"].astype(np.float32)
            + results[2 * b + 1]["partial"].astype(np.float32)
            + proj_b
        )
    return out


_NC_CACHE = {}


def _get_nc(T=FULL_T):
    if T not in _NC_CACHE:
        _NC_CACHE[T] = build_kernel(T)
    return _NC_CACHE[T]


def run(x, qkv_w, qkv_b, proj_w, proj_b, trace=False):
    nc = _get_nc()
    in_maps = shard_inputs(x, qkv_w, qkv_b, proj_w, proj_b)
    res = run_bass_kernel_spmd(nc, in_maps, list(range(N_CORES)), trace=trace)
    return combine_outputs(res.results, proj_b), res


def kernel(x, qkv_w, qkv_b, proj_w, proj_b):
    out, _ = run(x, qkv_w, qkv_b, proj_w, proj_b)
    return out



# revision 25
# speedup vs baseline: 1.0816x; 1.0816x over previous
"""Multi-head self-attention TRN2 kernel.

Full inputs -> shard over 8 NeuronCores as (batch b, head-group g):
core c = 2*b + g handles batch b and heads 8g..8g+7 (tensor parallel over
heads within a batch entry). Each core computes its heads' contribution to
the output projection; the host sums the two partials per batch and adds
proj bias.

Per-core pipeline (all matmuls bf16 with fp32 PSUM accumulation):
  x^T   via transposing DMA                      [C=1024, T=2048]
  Q^T,K^T = (w.T as lhsT) @ x^T  (+bias, DVE)    [512, T] col-major
  V     = (x^T as lhsT) @ wv (+bias via K=1 mm)  [T, 512] + ones col
  per head-pair hp, t-chunk of 1024, s-tile of 128:
    S^T chunk = K^T.T @ Q^T      (K=64 contraction, 2 heads row-tiled)
    P^T = exp(S^T/8)             ScalarE from PSUM, bf16 out, N=1024
    O  += P^T.T @ V_aug          (per 128-t subtile, N=65; col 64 = rowsum)
  normalize O by 1/rowsum (per-partition scalar), DMA-transpose -> O^T
  partial = O^T.T @ pw           [T, 1024] fp32 -> DRAM
"""

import numpy as np
import ml_dtypes
from contextlib import ExitStack

import concourse.bass as bass
import concourse.bacc as bacc
import concourse.tile as tile
from concourse import mybir
from concourse.bass_utils import run_bass_kernel_spmd

BF16 = mybir.dt.bfloat16
F32 = mybir.dt.float32
bf16 = ml_dtypes.bfloat16

P = 128
C = 1024          # hidden
HG = 8            # heads per core
D = 64            # head dim
DG = HG * D       # 512, per-core qkv width
N_CORES = 8
FULL_T = 2048
SCALE = D ** -0.5


def build_kernel(T=FULL_T):
    nc = bacc.Bacc(
        "TRN2", target_bir_lowering=False, debug=False, num_devices=N_CORES
    )
    x = nc.dram_tensor("x", [C, T], BF16, kind="ExternalInput").ap()
    wq = nc.dram_tensor("wq", [P, C // P, DG], BF16, kind="ExternalInput").ap()
    wk = nc.dram_tensor("wk", [P, C // P, DG], BF16, kind="ExternalInput").ap()
    wv = nc.dram_tensor("wv", [P, C // P, DG], BF16, kind="ExternalInput").ap()
    # cols 0..3 = q bias per col-tile, 4..7 = k bias
    bqk = nc.dram_tensor("bqk", [P, 8], F32, kind="ExternalInput").ap()
    bv = nc.dram_tensor("bv", [1, DG], BF16, kind="ExternalInput").ap()
    pw = nc.dram_tensor("pw", [P, DG // P, C], BF16, kind="ExternalInput").ap()
    partial = nc.dram_tensor("partial", [T, C], BF16, kind="ExternalOutput").ap()

    CT = C // P           # 8 contraction tiles over hidden
    TT = T // P           # t/s tiles of 128
    TCH = min(1024, T)    # t chunk width for attention (exp granularity)
    NCH = T // TCH        # number of t chunks
    SW = min(512, TCH)    # matmul moving-dim width (PSUM bank limit)
    NSW = TCH // SW       # sub-chunks per chunk
    Q8 = TCH // P         # 128-t subtiles per chunk
    KT4 = DG // P         # 4 col-tiles of Q^T/K^T/O^T

    with tile.TileContext(nc) as tc, ExitStack() as ctx:
        sb = ctx.enter_context(tc.tile_pool(name="sb", bufs=1))
        pdram = ctx.enter_context(tc.tile_pool(name="pdram", bufs=4, space="DRAM"))
        pon = ctx.enter_context(tc.tile_pool(name="pon", bufs=3))
        ppb = ctx.enter_context(tc.tile_pool(name="ppb", bufs=16))
        pout = ctx.enter_context(tc.tile_pool(name="pout", bufs=2))
        pp = ctx.enter_context(tc.tile_pool(name="pp", bufs=1, space="PSUM"))

        def o_tag(hx):
            return ("oA", "oB")[hx]

        # persistent SBUF tensors
        xT = sb.tile([P, CT, T], BF16)
        wq_s = sb.tile([P, CT, DG], BF16)
        wk_s = sb.tile([P, CT, DG], BF16)
        wv_s = sb.tile([P, CT, DG], BF16)
        pw_s = sb.tile([P, KT4, C], BF16)
        bqk_s = sb.tile([P, 8], F32)
        bvb_s = sb.tile([P, DG], BF16)
        QT = sb.tile([P, KT4, T], BF16)
        KTt = sb.tile([P, KT4, T], BF16)
        V = sb.tile([P, TT, HG, 65], BF16)
        OT = sb.tile([P, KT4, T], BF16)

        # ---- loads ----
        # x arrives pre-transposed from the host ([C, T]), so x^T loads are
        # straight DMAs — no xbar — split across both HWDGE queues. K/Q
        # weights first on the scalar queue (they gate the first pass).
        nc.scalar.dma_start(out=wk_s, in_=wk)
        nc.scalar.dma_start(out=wq_s, in_=wq)
        for ct in range(CT):
            nc.sync.dma_start(
                out=xT[:, ct, 0:TCH], in_=x[ct * P : (ct + 1) * P, 0:TCH]
            )
        for ct in range(CT):
            nc.scalar.dma_start(
                out=xT[:, ct, TCH:T], in_=x[ct * P : (ct + 1) * P, TCH:T]
            )
        nc.scalar.dma_start(out=bqk_s, in_=bqk)
        nc.scalar.dma_start(out=wv_s, in_=wv)
        nc.gpsimd.dma_start(out=bvb_s, in_=bv.to_broadcast((P, DG)))
        nc.gpsimd.dma_start(out=pw_s, in_=pw)
        nc.vector.memset(V[:, :, :, 64:65], 1.0)

        # ---- helpers ----
        # Fill work (V/QK/proj tiles) lives on a dedicated 2-bank psum tag
        # so it never blocks the attention chain's banks.
        def v_tile(tt):
            ps = pp.tile([P, DG], F32, tag="fill", name="psv")
            for ct in range(CT):
                nc.tensor.matmul(
                    ps,
                    lhsT=xT[:, ct, tt * P : (tt + 1) * P],
                    rhs=wv_s[:, ct, :],
                    start=(ct == 0),
                    stop=(ct == CT - 1),
                )
            # bias via DVE broadcast-add fused into the PSUM->SBUF copy
            nc.vector.tensor_add(
                V[:, tt, :, 0:64], ps.rearrange("p (h d) -> p h d", h=HG),
                bvb_s.rearrange("p (h d) -> p h d", h=HG),
            )

        def qk_tile(w_s, QKT, boff, i, th, tag="fill"):
            ps = pp.tile([P, TCH], F32, tag=tag, name="ps")
            for ct in range(CT):
                for nh in range(NSW):
                    nc.tensor.matmul(
                        ps[:, nh * SW : (nh + 1) * SW],
                        lhsT=w_s[:, ct, i * P : (i + 1) * P],
                        rhs=xT[
                            :, ct, th * TCH + nh * SW : th * TCH + (nh + 1) * SW
                        ],
                        start=(ct == 0),
                        stop=(ct == CT - 1),
                    )
            nc.vector.tensor_scalar_add(
                QKT[:, i, th * TCH : (th + 1) * TCH],
                ps,
                bqk_s[:, boff + i : boff + i + 1],
            )

        def proj_chunk(th, tags=("fill",)):
            for mt in range(th * (TT // NCH), (th + 1) * (TT // NCH)):
                ps_p = pp.tile([P, C], F32, tag=tags[mt % len(tags)], name="ps_p")
                for kk in range(KT4):
                    for nh in range(C // 512):
                        nc.tensor.matmul(
                            ps_p[:, nh * 512 : (nh + 1) * 512],
                            lhsT=OT[:, kk, mt * P : (mt + 1) * P],
                            rhs=pw_s[:, kk, nh * 512 : (nh + 1) * 512],
                            start=(kk == 0),
                            stop=(kk == KT4 - 1),
                        )
                ot = pout.tile([P, C], BF16, tag="ot", name="ot")
                nc.vector.tensor_copy(ot, ps_p)
                eng = nc.sync if mt % 2 == 0 else nc.scalar
                eng.dma_start(out=partial[mt * P : (mt + 1) * P, :], in_=ot)

        # ---- attention: single-stream software pipeline over 16 passes ----
        # One head-pass at a time; the S tiles double-buffer across two
        # psum tags (sA/sB by global step parity) so ScalarE never waits:
        # while exp(step) runs, PE computes S(step+1). PSUM: sA, sB
        # ([128, TCH] = 2 banks each) + oA accumulator ([65, TCH] = 2) = 6
        # banks; the remaining pair ("fill", 2 banks) hosts V/QK/proj tiles
        # which the scheduler slots into PE slack without ever touching the
        # chain's banks.
        passes = [
            (hp, th, hx)
            for hp in range(KT4)
            for th in range(NCH)
            for hx in range(2)
        ]

        def s_mm(hp, th, hx, st, parity):
            s_ps = pp.tile([P, TCH], F32, tag=("sA", "sB")[parity], name="s_ps")
            pr = slice(hx * 64, (hx + 1) * 64)
            for nh in range(NSW):
                nc.tensor.matmul(
                    s_ps[:, nh * SW : (nh + 1) * SW],
                    lhsT=KTt[pr, hp, st * P : (st + 1) * P],
                    rhs=QT[
                        pr, hp, th * TCH + nh * SW : th * TCH + (nh + 1) * SW
                    ],
                    start=True,
                    stop=True,
                )
            return s_ps

        def normalize(o_t, hp, th, hx):
            # copy O_aug^T out of PSUM first so the bank frees fast
            ou = pon.tile([65, TCH], F32, tag="ou", name="ou")
            nc.vector.tensor_copy(ou, o_t)
            eng0, eng1 = nc.sync, nc.scalar
            # 1/rowsum: bounce the [1,TCH] rowsum through DRAM into a
            # [128, TCH/128] layout so DVE reciprocal uses all lanes (a
            # [1,TCH] reciprocal costs 6.5us on one lane), then bounce
            # back -> stride-0 broadcast to 64 partitions
            rd = pdram.tile([1, TCH], F32, tag="rd", name="rd")
            eng0.dma_start(out=rd, in_=ou[64:65, :])
            rs = pon.tile([P, TCH // P], F32, tag="rs", name="rs")
            eng0.dma_start(
                out=rs, in_=rd.rearrange("a (p k) -> (a p) k", p=P)
            )
            rq = pon.tile([P, TCH // P], F32, tag="rq", name="rq")
            nc.vector.reciprocal(rq, rs)
            rqd = pdram.tile([1, TCH], F32, tag="rqd", name="rqd")
            eng1.dma_start(
                out=rqd.rearrange("a (p k) -> (a p) k", p=P), in_=rq
            )
            rb = pon.tile([64, TCH], F32, tag="rb", name="rb")
            eng1.dma_start(out=rb, in_=rqd.to_broadcast((64, TCH)))
            nc.vector.tensor_mul(
                OT[hx * 64 : (hx + 1) * 64, hp, th * TCH : (th + 1) * TCH],
                ou[0:64, :],
                rb,
            )

        # ---- emission ----
        # Fill work first (program order = dependency order: the chain
        # reads V/QT/KTt), interleaved in rough consumption order. The
        # chain itself is emitted afterwards under high_priority so the
        # scheduler prefers chain instructions whenever they are ready and
        # only slots fill work into PE idle moments.
        with tc.high_priority(offset=2 * 10**7):
            qk_tile(wk_s, KTt, 4, 0, 0)          # th0-half data only
            qk_tile(wq_s, QT, 0, 0, 0, tag="sB")  # parallel with KT0th0
        qk_tile(wk_s, KTt, 4, 0, 1)      # pass 0 needs s-tiles 8..15
        v_tile(0)
        v_tile(1)
        v_tile(2)
        v_tile(3)
        v_tile(4)
        v_tile(5)
        v_tile(6)
        v_tile(7)
        v_tile(8)
        v_tile(9)
        v_tile(10)
        v_tile(11)
        v_tile(12)
        v_tile(13)
        v_tile(14)
        v_tile(15)
        qk_tile(wq_s, QT, 0, 0, 1)       # passes 2-3 (hp=0, th=1) step 32
        qk_tile(wk_s, KTt, 4, 1, 0)      # passes 4-7 (hp=1) at step 64
        qk_tile(wk_s, KTt, 4, 1, 1)
        qk_tile(wq_s, QT, 0, 1, 0)
        qk_tile(wq_s, QT, 0, 1, 1)
        qk_tile(wk_s, KTt, 4, 2, 0)      # passes 8-11 (hp=2) at step 128
        qk_tile(wk_s, KTt, 4, 2, 1)
        qk_tile(wq_s, QT, 0, 2, 0)
        qk_tile(wq_s, QT, 0, 2, 1)
        qk_tile(wk_s, KTt, 4, 3, 0)      # passes 12-15 (hp=3) at step 192
        qk_tile(wk_s, KTt, 4, 3, 1)
        qk_tile(wq_s, QT, 0, 3, 0)
        qk_tile(wq_s, QT, 0, 3, 1)

        with tc.high_priority(offset=10**7):
            step = 0
            s_cur = s_mm(*passes[0], 0, 0)
            for pi, (hp, th, hx) in enumerate(passes):
                h = 2 * hp + hx
                o_t = pp.tile([65, TCH], F32, tag="oA", name="o_t")
                for st in range(TT):
                    pb = ppb.tile([P, TCH], BF16, tag="p", name="pb")
                    nc.scalar.activation(
                        out=pb,
                        in_=s_cur,
                        func=mybir.ActivationFunctionType.Exp,
                        scale=float(SCALE),
                    )
                    # O_aug^T += V_aug.T @ P^T (V stationary, P streams)
                    for nh in range(NSW):
                        nc.tensor.matmul(
                            o_t[:, nh * SW : (nh + 1) * SW],
                            lhsT=V[:, st, h, :],
                            rhs=pb[:, nh * SW : (nh + 1) * SW],
                            start=(st == 0),
                            stop=(st == TT - 1),
                        )
                    # prefetch next step's S (next st, or next pass's st=0)
                    step += 1
                    if st + 1 < TT:
                        s_cur = s_mm(hp, th, hx, st + 1, step % 2)
                    elif pi + 1 < len(passes):
                        s_cur = s_mm(*passes[pi + 1], 0, step % 2)
                normalize(o_t, hp, th, hx)

        proj_chunk(0, tags=("fill", "sA", "sB", "oA"))
        proj_chunk(1, tags=("fill", "sA", "sB", "oA"))

    nc.compile()
    return nc


def shard_inputs(x, qkv_w, qkv_b, proj_w, proj_b, T=FULL_T):
    """Build the 8 per-core input maps (host-side layout prep)."""
    x = np.asarray(x, dtype=np.float32)
    qkv_w = np.asarray(qkv_w, dtype=np.float32)
    qkv_b = np.asarray(qkv_b, dtype=np.float32)
    proj_w = np.asarray(proj_w, dtype=np.float32)
    in_maps = []
    for c in range(N_CORES):
        b, g = divmod(c, 2)
        sl = slice(g * DG, (g + 1) * DG)
        wqg = qkv_w[:, 0 * C + g * DG : 0 * C + (g + 1) * DG]
        wkg = qkv_w[:, 1 * C + g * DG : 1 * C + (g + 1) * DG]
        wvg = qkv_w[:, 2 * C + g * DG : 2 * C + (g + 1) * DG]
        bqg = qkv_b[0 * C + g * DG : 0 * C + (g + 1) * DG]
        bkg = qkv_b[1 * C + g * DG : 1 * C + (g + 1) * DG]
        bvg = qkv_b[2 * C + g * DG : 2 * C + (g + 1) * DG]
        pwg = proj_w[sl, :]

        def arr_w(w):  # [C, DG] -> [128, C//128, DG]
            return np.ascontiguousarray(
                w.reshape(C // P, P, DG).transpose(1, 0, 2)
            ).astype(bf16)

        bqk = np.ascontiguousarray(
            np.concatenate(
                [bqg.reshape(DG // P, P).T, bkg.reshape(DG // P, P).T], axis=1
            )
        ).astype(np.float32)
        in_maps.append(
            {
                "x": np.ascontiguousarray(x[b, :T].T).astype(bf16),
                "wq": arr_w(wqg),
                "wk": arr_w(wkg),
                "wv": arr_w(wvg),
                "bqk": bqk,
                "bv": np.ascontiguousarray(bvg[None, :]).astype(bf16),
                "pw": np.ascontiguousarray(
                    pwg.reshape(DG // P, P, C).transpose(1, 0, 2)
                ).astype(bf16),
            }
        )
    return in_maps


def combine_outputs(results, proj_b, T=FULL_T):
    proj_b = np.asarray(proj_b, dtype=np.float32)
    out = np.empty((N_CORES // 2, T, C), np.float32)
    for b in range(N_CORES // 2):
        out[b] = (
            results[2 * b]["partial"].astype(np.float32)
            + results[2 * b + 1]["partial"].astype(np.float32)
            + proj_b
        )
    return out


_NC_CACHE = {}


def _get_nc(T=FULL_T):
    if T not in _NC_CACHE:
        _NC_CACHE[T] = build_kernel(T)
    return _NC_CACHE[T]


def run(x, qkv_w, qkv_b, proj_w, proj_b, trace=False):
    nc = _get_nc()
    in_maps = shard_inputs(x, qkv_w, qkv_b, proj_w, proj_b)
    res = run_bass_kernel_spmd(nc, in_maps, list(range(N_CORES)), trace=trace)
    return combine_outputs(res.results, proj_b), res


def kernel(x, qkv_w, qkv_b, proj_w, proj_b):
    out, _ = run(x, qkv_w, qkv_b, proj_w, proj_b)
    return out



# revision 26
# speedup vs baseline: 1.0990x; 1.0161x over previous
"""Multi-head self-attention TRN2 kernel.

Full inputs -> shard over 8 NeuronCores as (batch b, head-group g):
core c = 2*b + g handles batch b and heads 8g..8g+7 (tensor parallel over
heads within a batch entry). Each core computes its heads' contribution to
the output projection; the host sums the two partials per batch and adds
proj bias.

Per-core pipeline (all matmuls bf16 with fp32 PSUM accumulation):
  x^T   via transposing DMA                      [C=1024, T=2048]
  Q^T,K^T = (w.T as lhsT) @ x^T  (+bias, DVE)    [512, T] col-major
  V     = (x^T as lhsT) @ wv (+bias via K=1 mm)  [T, 512] + ones col
  per head-pair hp, t-chunk of 1024, s-tile of 128:
    S^T chunk = K^T.T @ Q^T      (K=64 contraction, 2 heads row-tiled)
    P^T = exp(S^T/8)             ScalarE from PSUM, bf16 out, N=1024
    O  += P^T.T @ V_aug          (per 128-t subtile, N=65; col 64 = rowsum)
  normalize O by 1/rowsum (per-partition scalar), DMA-transpose -> O^T
  partial = O^T.T @ pw           [T, 1024] fp32 -> DRAM
"""

import numpy as np
import ml_dtypes
from contextlib import ExitStack

import concourse.bass as bass
import concourse.bacc as bacc
import concourse.tile as tile
from concourse import mybir
from concourse.bass_utils import run_bass_kernel_spmd

BF16 = mybir.dt.bfloat16
F32 = mybir.dt.float32
bf16 = ml_dtypes.bfloat16

P = 128
C = 1024          # hidden
HG = 8            # heads per core
D = 64            # head dim
DG = HG * D       # 512, per-core qkv width
N_CORES = 8
FULL_T = 2048
SCALE = D ** -0.5


def build_kernel(T=FULL_T):
    nc = bacc.Bacc(
        "TRN2", target_bir_lowering=False, debug=False, num_devices=N_CORES
    )
    x = nc.dram_tensor("x", [C, T], BF16, kind="ExternalInput").ap()
    wq = nc.dram_tensor("wq", [P, C // P, DG], BF16, kind="ExternalInput").ap()
    wk = nc.dram_tensor("wk", [P, C // P, DG], BF16, kind="ExternalInput").ap()
    wv = nc.dram_tensor("wv", [P, C // P, DG], BF16, kind="ExternalInput").ap()
    # cols 0..3 = q bias per col-tile, 4..7 = k bias
    bqk = nc.dram_tensor("bqk", [P, 8], F32, kind="ExternalInput").ap()
    bv = nc.dram_tensor("bv", [1, DG], BF16, kind="ExternalInput").ap()
    pw = nc.dram_tensor("pw", [P, DG // P, C], BF16, kind="ExternalInput").ap()
    partial = nc.dram_tensor("partial", [T, C], BF16, kind="ExternalOutput").ap()

    CT = C // P           # 8 contraction tiles over hidden
    TT = T // P           # t/s tiles of 128
    TCH = min(1024, T)    # t chunk width for attention (exp granularity)
    NCH = T // TCH        # number of t chunks
    SW = min(512, TCH)    # matmul moving-dim width (PSUM bank limit)
    NSW = TCH // SW       # sub-chunks per chunk
    Q8 = TCH // P         # 128-t subtiles per chunk
    KT4 = DG // P         # 4 col-tiles of Q^T/K^T/O^T

    with tile.TileContext(nc) as tc, ExitStack() as ctx:
        sb = ctx.enter_context(tc.tile_pool(name="sb", bufs=1))
        pdram = ctx.enter_context(tc.tile_pool(name="pdram", bufs=4, space="DRAM"))
        pon = ctx.enter_context(tc.tile_pool(name="pon", bufs=3))
        ppb = ctx.enter_context(tc.tile_pool(name="ppb", bufs=24))
        pout = ctx.enter_context(tc.tile_pool(name="pout", bufs=2))
        pp = ctx.enter_context(tc.tile_pool(name="pp", bufs=1, space="PSUM"))

        def o_tag(hx):
            return ("oA", "oB")[hx]

        # persistent SBUF tensors
        xT = sb.tile([P, CT, T], BF16)
        wq_s = sb.tile([P, CT, DG], BF16)
        wk_s = sb.tile([P, CT, DG], BF16)
        wv_s = sb.tile([P, CT, DG], BF16)
        pw_s = sb.tile([P, KT4, C], BF16)
        bqk_s = sb.tile([P, 8], F32)
        bvb_s = sb.tile([P, DG], BF16)
        QT = sb.tile([P, KT4, T], BF16)
        KTt = sb.tile([P, KT4, T], BF16)
        V = sb.tile([P, TT, HG, 65], BF16)
        OT = sb.tile([P, KT4, T], BF16)

        # ---- loads ----
        # x arrives pre-transposed from the host ([C, T]), so x^T loads are
        # straight DMAs — no xbar — split across both HWDGE queues. K/Q
        # weights first on the scalar queue (they gate the first pass).
        nc.scalar.dma_start(out=wk_s, in_=wk)
        nc.scalar.dma_start(out=wq_s, in_=wq)
        for ct in range(CT):
            nc.sync.dma_start(
                out=xT[:, ct, 0:TCH], in_=x[ct * P : (ct + 1) * P, 0:TCH]
            )
        for ct in range(CT):
            nc.scalar.dma_start(
                out=xT[:, ct, TCH:T], in_=x[ct * P : (ct + 1) * P, TCH:T]
            )
        nc.scalar.dma_start(out=bqk_s, in_=bqk)
        nc.scalar.dma_start(out=wv_s, in_=wv)
        nc.gpsimd.dma_start(out=bvb_s, in_=bv.to_broadcast((P, DG)))
        nc.gpsimd.dma_start(out=pw_s, in_=pw)
        nc.vector.memset(V[:, :, :, 64:65], 1.0)

        # ---- helpers ----
        # Fill work (V/QK/proj tiles) lives on a dedicated 2-bank psum tag
        # so it never blocks the attention chain's banks.
        def v_tile(tt):
            ps = pp.tile([P, DG], F32, tag="fill", name="psv")
            for ct in range(CT):
                nc.tensor.matmul(
                    ps,
                    lhsT=xT[:, ct, tt * P : (tt + 1) * P],
                    rhs=wv_s[:, ct, :],
                    start=(ct == 0),
                    stop=(ct == CT - 1),
                )
            # bias via DVE broadcast-add fused into the PSUM->SBUF copy
            nc.vector.tensor_add(
                V[:, tt, :, 0:64], ps.rearrange("p (h d) -> p h d", h=HG),
                bvb_s.rearrange("p (h d) -> p h d", h=HG),
            )

        def qk_tile(w_s, QKT, boff, i, th, tag="fill"):
            ps = pp.tile([P, TCH], F32, tag=tag, name="ps")
            for ct in range(CT):
                for nh in range(NSW):
                    nc.tensor.matmul(
                        ps[:, nh * SW : (nh + 1) * SW],
                        lhsT=w_s[:, ct, i * P : (i + 1) * P],
                        rhs=xT[
                            :, ct, th * TCH + nh * SW : th * TCH + (nh + 1) * SW
                        ],
                        start=(ct == 0),
                        stop=(ct == CT - 1),
                    )
            nc.vector.tensor_scalar_add(
                QKT[:, i, th * TCH : (th + 1) * TCH],
                ps,
                bqk_s[:, boff + i : boff + i + 1],
            )

        def proj_chunk(th, tags=("fill",)):
            for mt in range(th * (TT // NCH), (th + 1) * (TT // NCH)):
                ps_p = pp.tile([P, C], F32, tag=tags[mt % len(tags)], name="ps_p")
                for kk in range(KT4):
                    for nh in range(C // 512):
                        nc.tensor.matmul(
                            ps_p[:, nh * 512 : (nh + 1) * 512],
                            lhsT=OT[:, kk, mt * P : (mt + 1) * P],
                            rhs=pw_s[:, kk, nh * 512 : (nh + 1) * 512],
                            start=(kk == 0),
                            stop=(kk == KT4 - 1),
                        )
                ot = pout.tile([P, C], BF16, tag="ot", name="ot")
                nc.vector.tensor_copy(ot, ps_p)
                eng = nc.sync if mt % 2 == 0 else nc.scalar
                eng.dma_start(out=partial[mt * P : (mt + 1) * P, :], in_=ot)

        # ---- attention: single-stream software pipeline over 16 passes ----
        # One head-pass at a time; the S tiles double-buffer across two
        # psum tags (sA/sB by global step parity) so ScalarE never waits:
        # while exp(step) runs, PE computes S(step+1). PSUM: sA, sB
        # ([128, TCH] = 2 banks each) + oA accumulator ([65, TCH] = 2) = 6
        # banks; the remaining pair ("fill", 2 banks) hosts V/QK/proj tiles
        # which the scheduler slots into PE slack without ever touching the
        # chain's banks.
        passes = [
            (hp, th, hx)
            for hp in range(KT4)
            for th in range(NCH)
            for hx in range(2)
        ]

        def s_mm(hp, th, hx, st, parity):
            s_ps = pp.tile([P, TCH], F32, tag=("sA", "sB")[parity], name="s_ps")
            pr = slice(hx * 64, (hx + 1) * 64)
            for nh in range(NSW):
                nc.tensor.matmul(
                    s_ps[:, nh * SW : (nh + 1) * SW],
                    lhsT=KTt[pr, hp, st * P : (st + 1) * P],
                    rhs=QT[
                        pr, hp, th * TCH + nh * SW : th * TCH + (nh + 1) * SW
                    ],
                    start=True,
                    stop=True,
                )
            return s_ps

        def normalize(o_t, hp, th, hx):
            # copy O_aug^T out of PSUM first so the bank frees fast
            ou = pon.tile([65, TCH], F32, tag="ou", name="ou")
            nc.vector.tensor_copy(ou, o_t)
            eng0, eng1 = nc.sync, nc.scalar
            # 1/rowsum: bounce the [1,TCH] rowsum through DRAM into a
            # [128, TCH/128] layout so DVE reciprocal uses all lanes (a
            # [1,TCH] reciprocal costs 6.5us on one lane), then bounce
            # back -> stride-0 broadcast to 64 partitions
            rd = pdram.tile([1, TCH], F32, tag="rd", name="rd")
            eng0.dma_start(out=rd, in_=ou[64:65, :])
            rs = pon.tile([P, TCH // P], F32, tag="rs", name="rs")
            eng0.dma_start(
                out=rs, in_=rd.rearrange("a (p k) -> (a p) k", p=P)
            )
            rq = pon.tile([P, TCH // P], F32, tag="rq", name="rq")
            nc.vector.reciprocal(rq, rs)
            rqd = pdram.tile([1, TCH], F32, tag="rqd", name="rqd")
            eng1.dma_start(
                out=rqd.rearrange("a (p k) -> (a p) k", p=P), in_=rq
            )
            rb = pon.tile([64, TCH], F32, tag="rb", name="rb")
            eng1.dma_start(out=rb, in_=rqd.to_broadcast((64, TCH)))
            nc.vector.tensor_mul(
                OT[hx * 64 : (hx + 1) * 64, hp, th * TCH : (th + 1) * TCH],
                ou[0:64, :],
                rb,
            )

        # ---- emission ----
        # Fill work first (program order = dependency order: the chain
        # reads V/QT/KTt), interleaved in rough consumption order. The
        # chain itself is emitted afterwards under high_priority so the
        # scheduler prefers chain instructions whenever they are ready and
        # only slots fill work into PE idle moments.
        with tc.high_priority(offset=2 * 10**7):
            qk_tile(wk_s, KTt, 4, 0, 0)          # th0-half data only
            qk_tile(wq_s, QT, 0, 0, 0, tag="sB")  # parallel with KT0th0
        qk_tile(wk_s, KTt, 4, 0, 1)      # pass 0 needs s-tiles 8..15
        v_tile(0)
        v_tile(1)
        v_tile(2)
        v_tile(3)
        v_tile(4)
        v_tile(5)
        v_tile(6)
        v_tile(7)
        v_tile(8)
        v_tile(9)
        v_tile(10)
        v_tile(11)
        v_tile(12)
        v_tile(13)
        v_tile(14)
        v_tile(15)
        qk_tile(wq_s, QT, 0, 0, 1)       # passes 2-3 (hp=0, th=1) step 32
        qk_tile(wk_s, KTt, 4, 1, 0)      # passes 4-7 (hp=1) at step 64
        qk_tile(wk_s, KTt, 4, 1, 1)
        qk_tile(wq_s, QT, 0, 1, 0)
        qk_tile(wq_s, QT, 0, 1, 1)
        qk_tile(wk_s, KTt, 4, 2, 0)      # passes 8-11 (hp=2) at step 128
        qk_tile(wk_s, KTt, 4, 2, 1)
        qk_tile(wq_s, QT, 0, 2, 0)
        qk_tile(wq_s, QT, 0, 2, 1)
        qk_tile(wk_s, KTt, 4, 3, 0)      # passes 12-15 (hp=3) at step 192
        qk_tile(wk_s, KTt, 4, 3, 1)
        qk_tile(wq_s, QT, 0, 3, 0)
        qk_tile(wq_s, QT, 0, 3, 1)

        with tc.high_priority(offset=10**7):
            step = 0
            s_cur = s_mm(*passes[0], 0, 0)
            for pi, (hp, th, hx) in enumerate(passes):
                h = 2 * hp + hx
                o_t = pp.tile([65, TCH], F32, tag="oA", name="o_t")
                for st in range(TT):
                    pb = ppb.tile([P, TCH], BF16, tag="p", name="pb")
                    nc.scalar.activation(
                        out=pb,
                        in_=s_cur,
                        func=mybir.ActivationFunctionType.Exp,
                        scale=float(SCALE),
                    )
                    # O_aug^T += V_aug.T @ P^T (V stationary, P streams)
                    for nh in range(NSW):
                        nc.tensor.matmul(
                            o_t[:, nh * SW : (nh + 1) * SW],
                            lhsT=V[:, st, h, :],
                            rhs=pb[:, nh * SW : (nh + 1) * SW],
                            start=(st == 0),
                            stop=(st == TT - 1),
                        )
                    # prefetch next step's S (next st, or next pass's st=0)
                    step += 1
                    if st + 1 < TT:
                        s_cur = s_mm(hp, th, hx, st + 1, step % 2)
                    elif pi + 1 < len(passes):
                        s_cur = s_mm(*passes[pi + 1], 0, step % 2)
                normalize(o_t, hp, th, hx)

        proj_chunk(0, tags=("fill", "sA", "sB", "oA"))
        proj_chunk(1, tags=("fill", "sA", "sB", "oA"))

    nc.compile()
    return nc


def shard_inputs(x, qkv_w, qkv_b, proj_w, proj_b, T=FULL_T):
    """Build the 8 per-core input maps (host-side layout prep)."""
    x = np.asarray(x, dtype=np.float32)
    qkv_w = np.asarray(qkv_w, dtype=np.float32)
    qkv_b = np.asarray(qkv_b, dtype=np.float32)
    proj_w = np.asarray(proj_w, dtype=np.float32)
    in_maps = []
    for c in range(N_CORES):
        b, g = divmod(c, 2)
        sl = slice(g * DG, (g + 1) * DG)
        wqg = qkv_w[:, 0 * C + g * DG : 0 * C + (g + 1) * DG]
        wkg = qkv_w[:, 1 * C + g * DG : 1 * C + (g + 1) * DG]
        wvg = qkv_w[:, 2 * C + g * DG : 2 * C + (g + 1) * DG]
        bqg = qkv_b[0 * C + g * DG : 0 * C + (g + 1) * DG]
        bkg = qkv_b[1 * C + g * DG : 1 * C + (g + 1) * DG]
        bvg = qkv_b[2 * C + g * DG : 2 * C + (g + 1) * DG]
        pwg = proj_w[sl, :]

        def arr_w(w):  # [C, DG] -> [128, C//128, DG]
            return np.ascontiguousarray(
                w.reshape(C // P, P, DG).transpose(1, 0, 2)
            ).astype(bf16)

        bqk = np.ascontiguousarray(
            np.concatenate(
                [bqg.reshape(DG // P, P).T, bkg.reshape(DG // P, P).T], axis=1
            )
        ).astype(np.float32)
        in_maps.append(
            {
                "x": np.ascontiguousarray(x[b, :T].T).astype(bf16),
                "wq": arr_w(wqg),
                "wk": arr_w(wkg),
                "wv": arr_w(wvg),
                "bqk": bqk,
                "bv": np.ascontiguousarray(bvg[None, :]).astype(bf16),
                "pw": np.ascontiguousarray(
                    pwg.reshape(DG // P, P, C).transpose(1, 0, 2)
                ).astype(bf16),
            }
        )
    return in_maps


def combine_outputs(results, proj_b, T=FULL_T):
    proj_b = np.asarray(proj_b, dtype=np.float32)
    out = np.empty((N_CORES // 2, T, C), np.float32)
    for b in range(N_CORES // 2):
        out[b] = (
            results[2 * b]["partial"].astype(np.float32)
            + results[2 * b + 1]["partial"].astype(np.float32)
            + proj_b
        )
    return out


_NC_CACHE = {}


def _get_nc(T=FULL_T):
    if T not in _NC_CACHE:
        _NC_CACHE[T] = build_kernel(T)
    return _NC_CACHE[T]


def run(x, qkv_w, qkv_b, proj_w, proj_b, trace=False):
    nc = _get_nc()
    in_maps = shard_inputs(x, qkv_w, qkv_b, proj_w, proj_b)
    res = run_bass_kernel_spmd(nc, in_maps, list(range(N_CORES)), trace=trace)
    return combine_outputs(res.results, proj_b), res


def kernel(x, qkv_w, qkv_b, proj_w, proj_b):
    out, _ = run(x, qkv_w, qkv_b, proj_w, proj_b)
    return out

